# revision 16
# baseline (speedup 1.0000x reference)
"""Causal multi-head self-attention on 8 Trainium2 NeuronCores.

Sharding: tensor-parallel over heads. 16 heads / 8 cores = 2 heads per core.
Each core computes the QKV projection for its 2 heads (full sequence, both
batches), causal flash-style attention for its 2 heads, and a partial output
projection against its slice of W_o columns. The host sums the 8 partial
outputs (the "all-reduce" of the tensor-parallel scheme, done during unshard).

Matmul inputs are fp16 (1 row/cycle on the PE, 11 mantissa bits), accumulation
is fp32 in PSUM, softmax in fp32. Output partials are written fp16 (halves
output DMA; adds ~1e-4 error against a 2e-2 budget).

The PE runs at 2.4 GHz only after ~3.4us of *continuous* execution and drops
to 1.2 GHz after any ~3.4us idle gap (HAM clock gate), so the emission order
keeps the tensor queue dense end to end:
  - input DMAs split across the sync and scalar HWDGE queues; the first x
    half-block + Q weights lead so the projection starts as early as possible,
    and ~24 warm-up matmuls on the Q weights warm the HAM gate while the
    first x block is still in flight.
  - batch-1's projection/V-transpose groups are emitted as PE fillers inside
    batch-0's attention chunks (attention is exp/ACT-gated, leaving PE slack),
    batch-0's output-projection tiles fill batch-0/1 attention slack, and
    batch-1's own out-proj tiles fill batch-1's attention.
  - the AV matmuls lag the score/exp stream by LAG blocks so a PSUM-free wait
    never head-of-line-blocks the in-order PE queue.
  - the final 512 queries of batch 1 run as four independent 128-token
    chunks, scores/exp for all four emitted first, then the AV chains, so
    the PE never stalls on a single exp at the very end (where no fillers
    remain and an idle gap would re-throttle the clock to 1.2 GHz).

Engine budget balancing (exp keeps ACT ~35us/batch busy during attention):
  - the causal mask is a 0/1 triangular DVE multiply on the exp output
    (off the PE); score matmuls are single-shot fp32.
  - batch-0 proj PSUM->SBUF copies on ACT/DVE (ACT idle during proj phase),
    batch-1's (emitted during batch-0 attention) all on DVE: exp owns ACT.
  - normalize: GPSIMD partition-broadcast of the PSUM row-sum row, in-place
    ~51ULP reciprocal, one DVE multiply straight from PSUM into mhaT fp16.
  - out staging fp32 PSUM -> fp16 SBUF on DVE mid-stream, ACT at the tail
    where it is idle.

Device layout (contraction dim always on partitions):
  - x pre-transposed/pre-cast on host: xT [D, B*S] fp16.
  - Projection computes Q^T/K^T/V^T [128=2*dk, S] per batch.
  - Scores transposed, S^T[k, q] = K^T.T @ Q^T, both heads in one
    [128, 2, 512] fp32 PSUM tile via separate PE row groups (concurrent
    strips).
  - One ACT exp per score tile (PSUM -> SBUF fp16); the diagonal 128-band
    is zeroed post-exp by a triangular 0/1 DVE multiply; fully-masked
    columns are never computed.
  - V^T transposed on-PE to V[tok, dv] with a ones column appended, so the
    AV matmul also accumulates softmax row-sums (row 64 of the output).
"""

from collections import deque

import numpy as np

import concourse.bacc as bacc
import concourse.mybir as mybir
import concourse.tile as tile

FP32 = mybir.dt.float32
FP16 = mybir.dt.float16

B = 2
S = 2048
D = 1024
NUM_HEADS = 16
DK = 64
NCORES = 8
HPC = NUM_HEADS // NCORES  # heads per core = 2
HD = HPC * DK  # 128, head dims per core

QCW = 512  # q chunk width
KTW = 128  # k tile width (partition dim)

NP_IN = np.float16


def build_nc(d=D, s=S, b=B):
    """Build the per-core Bass program. All 8 cores run this same program."""
    assert d % 128 == 0 and s % QCW == 0 and QCW % KTW == 0
    ndc = d // 128  # d_model chunks (8)
    nqc = s // QCW  # q chunks per batch (4)
    nkt = s // KTW  # k tiles per batch (16)
    kpq = QCW // KTW  # k tiles per q chunk (4)
    ntt = s // 128  # token tiles per batch (16)
    tpq = ntt // nqc  # out-proj token tiles per q chunk (4)

    nc = bacc.Bacc("TRN2", target_bir_lowering=False)

    # x packed on host as [128, b, nqc, ndc, QCW] so one DMA per (batch,
    # q-block) moves 2MB in 4KB-contiguous per-partition runs
    xT_d = nc.dram_tensor(
        "xT", [128, b * (s // QCW) * ndc * QCW], FP16, kind="ExternalInput"
    )
    # wqkvT packed as [128, 3(m), ndc, 128] with contiguous per-partition
    # runs: one DMA per projection matrix, Q's weights land first
    wt_d = nc.dram_tensor("wqkvT", [128, 3 * ndc * 128], FP16, kind="ExternalInput")
    wo_d = nc.dram_tensor("woT", [HD, d], FP16, kind="ExternalInput")
    tri_d = nc.dram_tensor("tri", [128, 2 * 128], FP16, kind="ExternalInput")
    id_d = nc.dram_tensor("ident", [128, 128], FP16, kind="ExternalInput")
    out_d = nc.dram_tensor("out", [b * s, d], FP16, kind="ExternalOutput")

    with tile.TileContext(nc) as tc:
        with (
            tc.tile_pool(name="consts", bufs=1) as consts,
            tc.tile_pool(name="xts", bufs=b * (s // QCW)) as xts_pool,
            tc.tile_pool(name="qkv", bufs=2) as qkv_pool,
            tc.tile_pool(name="vsb", bufs=2) as v_pool,
            tc.tile_pool(name="pt", bufs=6) as pt_pool,
            tc.tile_pool(name="pt128", bufs=10) as pt128_pool,
            tc.tile_pool(name="mha", bufs=2) as mha_pool,
            tc.tile_pool(name="osb", bufs=3) as out_pool,
            tc.tile_pool(name="small", bufs=2) as small_pool,
            tc.tile_pool(name="ps_mm", bufs=2, space="PSUM") as ps_mm,
            tc.tile_pool(name="ps_s", bufs=2, space="PSUM") as ps_s,
            tc.tile_pool(name="ps_o", bufs=1, space="PSUM") as ps_o,
        ):
            # ---- input DMA, split across the two HWDGE queues (sync +
            # scalar) so dispatch overlaps; ordered so the first projection
            # group can start as early as possible: first x half-block + Q
            # weights lead, K/V weights interleave between x pieces.
            wt_sb = consts.tile([128, 3, ndc, 128], FP16)
            xblk = [
                [
                    xts_pool.tile(
                        [128, ndc, QCW], FP16, name=f"xb{bi}_{n}", tag="xt"
                    )
                    for n in range(nqc)
                ]
                for bi in range(b)
            ]

            def dma_xblock(bi, n, eng=None):
                base = (bi * nqc + n) * ndc * QCW
                (eng or nc.sync).dma_start(
                    xblk[bi][n], xT_d[:, base : base + ndc * QCW]
                )

            def dma_w(m):
                nc.sync.dma_start(
                    wt_sb[:, m, :, :],
                    wt_d[:, m * ndc * 128 : (m + 1) * ndc * 128],
                )

            dma_w(0)  # Q weights first: warm-up matmuls + first proj group
            # first x block in halves so proj k=0..3 can start ~3us earlier
            nc.sync.dma_start(
                xblk[0][0][:, 0 : ndc // 2, :], xT_d[:, 0 : (ndc // 2) * QCW]
            )
            dma_w(1)
            nc.sync.dma_start(
                xblk[0][0][:, ndc // 2 :, :],
                xT_d[:, (ndc // 2) * QCW : ndc * QCW],
            )
            dma_w(2)
            # tiny consts next (negligible delay to the x stream), then
            # the rest of batch-0's x, wo, and batch-1's x: HWDGE is FIFO
            # per engine, so the transfers pace exactly in this order
            tri_sb = consts.tile([128, 2, 128], FP16)
            nc.sync.dma_start(tri_sb, tri_d[:, :])
            id_sb = consts.tile([128, 128], FP16)
            nc.sync.dma_start(id_sb, id_d[:, :])
            for n in range(1, nqc):
                dma_xblock(0, n)
            wo_sb = consts.tile([128, d], FP16)
            nc.sync.dma_start(wo_sb, wo_d[:, :])
            for n in range(nqc):
                dma_xblock(1, n)

            qkvTs = [qkv_pool.tile([128, 3, s], FP16, name=f"qkvT{bi}", tag="qkvT")
                     for bi in range(b)]
            v_sbs = [v_pool.tile([128, nkt, 2 * (DK + 1)], FP16, name=f"v{bi}",
                                 tag="vsb") for bi in range(b)]
            mhaTs = [mha_pool.tile([128, s], FP16, name=f"mhaT{bi}", tag="mhaT")
                     for bi in range(b)]
            for bi in range(b):
                nc.gpsimd.memset(v_sbs[bi], 1.0)

            # ---- HAM warm-up: ~24 throwaway matmuls on the Q weights while
            # the first x block is still in flight. Keeps the PE busy from
            # ~9.5us so the 2.4 GHz clock gate is open when real work lands.
            wu = ps_mm.tile([128, 128], FP32, name="wu", tag="mm")
            for _ in range(24):
                nc.tensor.matmul(
                    wu, wt_sb[:, 0, 0, :], wt_sb[:, 0, 0, :],
                    start=True, stop=True,
                )

            def emit_proj_group(bi, m, n, copy_eng):
                qkvT = qkvTs[bi]
                pp = ps_mm.tile([128, QCW], FP32, name="pp", tag="mm")
                for k in range(ndc):
                    nc.tensor.matmul(
                        pp,
                        wt_sb[:, m, k, :],
                        xblk[bi][n][:, k, :],
                        start=(k == 0),
                        stop=(k == ndc - 1),
                    )
                dst = qkvT[:, m, QCW * n : QCW * (n + 1)]
                if copy_eng == "act":
                    nc.scalar.copy(dst, pp)
                else:
                    nc.vector.tensor_copy(dst, pp)

            def emit_trans(bi, t):
                qkvT, v_sb = qkvTs[bi], v_sbs[bi]
                tp = ps_mm.tile([128, 128], FP16, name="tp", tag="mm")
                nc.tensor.transpose(tp, qkvT[:, 2, 128 * t : 128 * (t + 1)], id_sb)
                nc.vector.tensor_copy(v_sb[:, t, 0:DK], tp[:, 0:DK])
                nc.vector.tensor_copy(
                    v_sb[:, t, DK + 1 : 2 * DK + 1], tp[:, DK : 2 * DK]
                )

            def emit_scores(bi, q0, w, kt, spool=ps_s, ptpool=pt_pool, pw=None):
                """Score matmuls + exp (+ causal mask) for one k tile.

                Returns the fp16 exp tile and its column offset. The causal
                mask is applied post-exp as a 0/1 triangular DVE multiply on
                the diagonal 128-block (off the PE), so the score matmuls
                stay single-shot.
                """
                qkvT = qkvTs[bi]
                band = KTW * kt >= q0
                c0 = KTW * kt - q0 if band else 0
                pw = pw or QCW
                # sp stays full width: the two heads' concurrent row-strip
                # matmuls must land in SEPARATE PSUM banks (2KB per head)
                sp = spool.tile([128, 2, QCW], FP32, name="sp", tag="s")
                nc.tensor.matmul(
                    sp[:, 0, c0:w],
                    qkvT[0:DK, 1, KTW * kt : KTW * (kt + 1)],
                    qkvT[0:DK, 0, q0 + c0 : q0 + w],
                    start=True,
                    stop=True,
                )
                nc.tensor.matmul(
                    sp[:, 1, c0:w],
                    qkvT[DK : 2 * DK, 1, KTW * kt : KTW * (kt + 1)],
                    qkvT[DK : 2 * DK, 0, q0 + c0 : q0 + w],
                    start=True,
                    stop=True,
                )
                pt = ptpool.tile([128, 2, pw], FP16, name="pt", tag="pt")
                nc.scalar.activation(
                    pt[:, :, c0:w],
                    sp[:, :, c0:w],
                    mybir.ActivationFunctionType.Exp,
                )
                if band:
                    # zero the upper-triangular (masked) part of the
                    # diagonal 128-block for both heads in one multiply.
                    # GPSIMD (otherwise idle, SBUF-only operands) keeps this
                    # off the DVE queue, whose in-order drain would delay
                    # the normalize that releases the attention PSUM banks.
                    nc.gpsimd.tensor_mul(
                        pt[:, :, c0 : c0 + KTW],
                        pt[:, :, c0 : c0 + KTW],
                        tri_sb,
                    )
                return pt, c0

            def emit_av(bi, oA, oB, pt, c0, w, kt, kts):
                v_sb = v_sbs[bi]
                nc.tensor.matmul(
                    oA[:, c0:w],
                    v_sb[:, kt, 0 : DK + 1],
                    pt[:, 0, c0:w],
                    start=(kt == 0),
                    stop=(kt == kts - 1),
                )
                nc.tensor.matmul(
                    oB[:, c0:w],
                    v_sb[:, kt, DK + 1 : 2 * DK + 2],
                    pt[:, 1, c0:w],
                    start=(kt == 0),
                    stop=(kt == kts - 1),
                )

            def emit_normalize(bi, oA, oB, q0, w):
                # ~51ULP reciprocal of the PSUM row-sum row staged to a
                # base-0 SBUF row (DVE handles the partition shift), gpsimd
                # broadcast, one DVE multiply from PSUM into mhaT
                mhaT = mhaTs[bi]
                for h, oh in ((0, oA), (1, oB)):
                    rs = small_pool.tile([1, w], FP32, name="rs", tag=f"rs{h}")
                    nc.vector.tensor_copy(rs, oh[DK : DK + 1, :])
                    nc.vector.reciprocal_approx_fast(out=rs, in_=rs)
                    bc = small_pool.tile([DK, w], FP32, name="bc", tag=f"bc{h}")
                    nc.gpsimd.partition_broadcast(bc, rs, channels=DK)
                    nc.vector.tensor_mul(
                        mhaT[DK * h : DK * (h + 1), q0 : q0 + w], oh[0:DK, :], bc
                    )

            def emit_attn_chunk(bi, q0, w, fillers):
                """Attention for batch bi, queries [q0, q0+w).

                The AV matmuls lag the score/exp stream by LAG blocks; one
                filler (independent PE work) drains per lagged slot.
                """
                oA = ps_o.tile([DK + 1, w], FP32, name="oA", tag="oA")
                oB = ps_o.tile([DK + 1, w], FP32, name="oB", tag="oB")
                kts = (q0 + w) // KTW
                # LAG=4: the AV stream trails the score/exp stream by 4 k
                # tiles, so (a) an ACT backlog never head-of-line-blocks the
                # PE on a missing exp, and (b) the first AV of a chunk lands
                # ~2us after the chunk starts, absorbing the WAR wait on the
                # previous chunk's normalize (o banks are single-buffered).
                LAG = 4
                pts = {}
                for i in range(kts + LAG):
                    if i < kts:
                        pts[i] = emit_scores(bi, q0, w, i)
                    if i >= LAG:
                        kt = i - LAG
                        pt, c0 = pts.pop(kt)
                        emit_av(bi, oA, oB, pt, c0, w, kt, kts)
                        if fillers:
                            fillers.popleft()[1]()
                emit_normalize(bi, oA, oB, q0, w)

            def emit_fp_tile(bi, t, cast_act=False, split_dma=False):
                mhaT = mhaTs[bi]
                ob = out_pool.tile([128, d], FP16, name="ob", tag="ob")
                r0 = bi * s + 128 * t
                for half in range(d // QCW):
                    fp = ps_mm.tile([128, QCW], FP32, name="fp", tag="mm")
                    nc.tensor.matmul(
                        fp,
                        mhaT[:, 128 * t : 128 * (t + 1)],
                        wo_sb[:, QCW * half : QCW * (half + 1)],
                    )
                    dst = ob[:, QCW * half : QCW * (half + 1)]
                    # mid-stream both halves go to DVE (ACT is exp-saturated
                    # and would hold the PSUM buf); at the tail ACT is idle
                    # so one half goes there and the copies run in parallel
                    if cast_act and half == 0:
                        nc.scalar.copy(dst, fp)
                    else:
                        nc.vector.tensor_copy(dst, fp)
                    if split_dma:
                        nc.sync.dma_start(
                            out_d[r0 : r0 + 128, QCW * half : QCW * (half + 1)],
                            dst,
                        )
                if not split_dma:
                    nc.sync.dma_start(out_d[r0 : r0 + 128, :], ob)

            # ---- batch-0 projection, n-block ordered behind the DMA stream
            for n in range(nqc):
                emit_proj_group(0, 0, n, "act")
                emit_proj_group(0, 1, n, "act")
                emit_proj_group(0, 2, n, "dve")
                for t in range(kpq * n, kpq * (n + 1)):
                    emit_trans(0, t)

            # batch-1 projection/transpose units, consumed as PE fillers
            # inside batch-0's attention (copies off ACT: exp owns it there)
            b1u = []
            for n in range(nqc):
                b1u.append(lambda n=n: emit_proj_group(1, 0, n, "act"))
                b1u.append(lambda n=n: emit_proj_group(1, 1, n, "act"))
                b1u.append(lambda n=n: emit_proj_group(1, 2, n, "dve"))
                b1u.append(
                    lambda n=n: [
                        emit_trans(1, t) for t in range(kpq * n, kpq * (n + 1))
                    ]
                )

            fill = deque()
            for qc in range(nqc):
                emit_attn_chunk(0, QCW * qc, QCW, fill)
                for t in range(tpq * qc, tpq * (qc + 1)):
                    fill.append(("fp", lambda t=t: emit_fp_tile(0, t)))
                take, b1u = b1u[:6], b1u[6:]
                for u in take:
                    fill.append(("b1u", u))
            # batch-1 attention reads qkvT[1]/v_sb[1]: every pending batch-1
            # projection unit must be emitted before its first score matmul.
            # Leftover out-proj tiles stay queued as b1-attention fillers.
            for u in b1u:
                u()
            for kind, fn in [e for e in fill if e[0] == "b1u"]:
                fn()
            fill = deque(e for e in fill if e[0] == "fp")

            # ---- batch-1 attention. The final-ordering trick: the four
            # SMALL chunks (queries 0-511, shallow kt depth, serial
            # exp->mask->AV->normalize chains) run FIRST, while the big
            # chunks that follow provide dense PE work to hide those
            # chains. The kernel then ENDS with the deepest chunk (q0=1536,
            # 16 back-to-back AV pairs): its exp stream completed long
            # before, so the PE runs gap-free to the last out-proj tile and
            # the clock gate stays open.
            tail = [384, 256, 128, 0]
            tail_pts = {}
            for q0 in tail:
                kts = q0 // KTW + 1
                tail_pts[q0] = [
                    emit_scores(1, q0, 128, kt, ptpool=pt128_pool, pw=128)
                    for kt in range(kts)
                ]
                if fill:
                    fill.popleft()[1]()
            oAll = [
                ps_o.tile([DK + 1, 4, 128], FP32, name=f"oT{h}", tag=t)
                for h, t in ((0, "oA"), (1, "oB"))
            ]
            for c, q0 in enumerate(tail):
                kts = q0 // KTW + 1
                for kt, (pt, c0) in enumerate(tail_pts[q0]):
                    for h in range(2):
                        nc.tensor.matmul(
                            oAll[h][:, c, :],
                            v_sbs[1][:, kt, h * (DK + 1) : (h + 1) * (DK + 1)],
                            pt[:, h, :],
                            start=(kt == 0),
                            stop=(kt == kts - 1),
                        )
                if fill:
                    fill.popleft()[1]()
            # normalizes in reverse emission order: the first read of the
            # packed o banks then depends on the LAST small-cluster AV, and
            # the in-order DVE queue keeps every later read collision-free
            # (PE writes to other columns of a bank a DVE read targets are
            # fatal, and Tile dependency tracking is range- not bank-level).
            # The out-proj tiles go into the filler queue: the big chunks'
            # AV slots consume them once their mhaT rows are final.
            for c, q0 in reversed(list(enumerate(tail))):
                mhaT = mhaTs[1]
                for h in range(2):
                    rs = small_pool.tile([1, 128], FP32, name="rs", tag=f"rs{h}")
                    nc.vector.tensor_copy(rs, oAll[h][DK : DK + 1, c, :])
                    nc.vector.reciprocal_approx_fast(out=rs, in_=rs)
                    bc = small_pool.tile([DK, 128], FP32, name="bc", tag=f"bc{h}")
                    nc.gpsimd.partition_broadcast(bc, rs, channels=DK)
                    nc.vector.tensor_mul(
                        mhaT[DK * h : DK * (h + 1), q0 : q0 + 128],
                        oAll[h][0:DK, c, :],
                        bc,
                    )
                fill.append(
                    ("fp", lambda t=q0 // 128: emit_fp_tile(1, t))
                )

            for q0 in (512, 1024, 1536):
                emit_attn_chunk(1, q0, QCW, fill)
                if q0 != 1536:
                    for t in range(q0 // 128, q0 // 128 + tpq):
                        fill.append(
                            ("fp", lambda t=t: emit_fp_tile(1, t))
                        )
            while fill:
                fill.popleft()[1]()
            for t in range(12, 16):
                emit_fp_tile(1, t, cast_act=True, split_dma=(t == 15))

    nc.compile()
    return nc


def make_core_inputs(x, W_qkv, W_o, d=D, s=S, b=B):
    """Host-side shard prep. Returns list of per-core input dicts."""
    nh = W_qkv.shape[0] // (3 * DK)
    ndc, nqc = d // 128, s // QCW
    xT = x.astype(np.float32).transpose(2, 0, 1).reshape(d, b * s).astype(NP_IN)
    # pack to [128, b, nqc, ndc, QCW]: per (batch, q-block) DMA, partition p
    # holds the 8 d-chunk rows back to back (4KB contiguous runs)
    xT = np.ascontiguousarray(
        xT.reshape(ndc, 128, b, nqc, QCW)
        .transpose(1, 2, 3, 0, 4)
        .reshape(128, b * nqc * ndc * QCW)
    )
    # tri[k,q] = 1 where q>=k (keep), 0 where q<k: multiplied into the exp
    # output on DVE so masked positions contribute nothing to AV/row-sums.
    # Duplicated side by side so one multiply masks both heads' slices.
    tri1 = np.where(
        np.triu(np.ones((128, 128), dtype=bool)), 1, 0
    ).astype(NP_IN)
    tri = np.ascontiguousarray(np.concatenate([tri1, tri1], axis=1))
    ident = np.eye(128, dtype=NP_IN)
    scale = np.float32(1.0 / np.sqrt(DK))
    in_maps = []
    for c in range(NCORES):
        h0 = HPC * c
        r = slice(h0 * DK, (h0 + HPC) * DK)
        wq = W_qkv[0 * nh * DK :][r] * scale
        wk = W_qkv[1 * nh * DK :][r]
        wv = W_qkv[2 * nh * DK :][r]
        ws = np.concatenate([wq, wk, wv], axis=0)  # [3*HD, d]
        # [d, 3*HD] -> packed [128, 3(m), ndc, 128] m-major
        wT = np.ascontiguousarray(
            ws.T.astype(NP_IN)
            .reshape(ndc, 128, 3, 128)
            .transpose(1, 2, 0, 3)
            .reshape(128, 3 * ndc * 128)
        )
        woT = np.ascontiguousarray(W_o[:, r].T.astype(NP_IN))  # [HD, d]
        in_maps.append(
            {"xT": xT, "wqkvT": wT, "woT": woT, "tri": tri, "ident": ident}
        )
    return in_maps


_NC_CACHE = {}


def kernel(x, W_qkv, W_o):
    from concourse.bass_utils import run_bass_kernel_spmd

    b, s, d = x.shape
    if "nc" not in _NC_CACHE:
        _NC_CACHE["nc"] = build_nc(d=d, s=s, b=b)
    nc = _NC_CACHE["nc"]
    in_maps = make_core_inputs(x, W_qkv, W_o, d=d, s=s, b=b)
    res = run_bass_kernel_spmd(nc, in_maps, core_ids=list(range(NCORES)))
    out = res.results[0]["out"].astype(np.float64)
    for c in range(1, NCORES):
        out += res.results[c]["out"]
    return out.astype(np.float32).reshape(b, s, d)
